# revision 29
# baseline (speedup 1.0000x reference)
"""Multi-head attention (B=2, L=2048, H=16, D=64) on 8 TRN2 NeuronCores.

Sharding: core = (batch b, head-group hg); 2 batches x 4 groups of 4 heads.
All matmul inputs bf16 (cast host-side), PSUM f32. rel_fro ~4.6e-3.

Dataflow per core (batch b, 4 heads = 2 pairs m, heads hl in pair):
    Q^T/K^T = W^T x^T        (d on partitions; head 2m at rows 0:64,
                              head 2m+1 at rows 64:128)
    V       = x W_v          (j on partitions, + ones column for denom)
    S^T     = K^T.T Q^T      (j on partitions, i free)
    es      = exp(S^T/8)     (bf16)
    O'^T    = [V|1].T es     (row 64 = denominator)
    O^T     = O'[0:64] * (1/O'[64])
    outT_kk = Wo[m-rows]^T O^T   (per-pair partials; host sums kk=0,1
                                  partials along with the 4 head-groups)

Schedule: attention runs as 8 per-head units (ih, m, hl), each with its own
[65,1024] O accumulator (po bufs=1, 2 banks) and a 3-deep S ring
(ps bufs=3, 6 banks). The ACT engine (exp, ~1.15us per [128,1024] tile)
is the attention bottleneck; the PE's slack absorbs, via transient borrows
of the third S buffer: the m=1 QK projection (units 0-1) and the output
projection, split into 32 (ih, kk, ct) half-chunks of 2 matmuls each,
emitted as soon as their oT half is normalized. Only the (ih1, kk1)
quarter remains after the last unit, so the tail is ~1 normalize +
8 half-chunks + a 2MB bf16 output flush.
"""

import sys

try:
    import concourse.bass as bass  # noqa: F401
except ImportError:  # pragma: no cover - path fallback
    sys.path.insert(0, "/opt/trn_rl_repo")

import numpy as np
import ml_dtypes
import concourse.bass as bass
import concourse.mybir as mybir
import concourse.tile as tile
from concourse import bacc
from concourse.bass_utils import run_bass_kernel_spmd

F32 = mybir.dt.float32
BF16 = mybir.dt.bfloat16
AF = mybir.ActivationFunctionType
NPBF16 = ml_dtypes.bfloat16

B = 2
L = 2048          # sequence length
C = 1024          # model dim
H_LOC = 4         # heads per core
D = 64            # head dim
HD = H_LOC * D    # 256 = local head-group width
KT = C // 128     # 8 k-tiles over the model dim
SCALE2 = float(D) ** -0.5  # 1/8, applied once inside exp

_cache = {}


def _build():
    nc = bacc.Bacc("TRN2", target_bir_lowering=False, debug=False, num_devices=8)

    # all inputs host-prearranged to the SBUF layout (contiguous DMA)
    xT = nc.declare_dram_parameter("xT", [128, KT * L], BF16, isOutput=False)
    wq = nc.declare_dram_parameter("wq", [128, KT * HD], BF16, isOutput=False)
    wk = nc.declare_dram_parameter("wk", [128, KT * HD], BF16, isOutput=False)
    wv = nc.declare_dram_parameter("wv", [128, KT * HD], BF16, isOutput=False)
    wo = nc.declare_dram_parameter("wo", [128, 2 * C], BF16, isOutput=False)
    outT0 = nc.declare_dram_parameter("outT0", [C, L], BF16, isOutput=True)
    outT1 = nc.declare_dram_parameter("outT1", [C, L], BF16, isOutput=True)
    outTs = (outT0, outT1)

    with tile.TileContext(nc) as tc:
        with tc.tile_pool(name="sb", bufs=1) as sb:

            # ---- load inputs (wq/wk first so projections start early) ------
            wq_sb = sb.tile([128, KT, HD], BF16, tag="wq")
            wk_sb = sb.tile([128, KT, HD], BF16, tag="wk")
            wv_sb = sb.tile([128, KT, HD], BF16, tag="wv")
            xT_sb = sb.tile([128, KT, L], BF16, tag="xT")
            nc.sync.dma_start(wq_sb[:, :, :], wq.rearrange("p (k c) -> p k c", k=KT))
            nc.sync.dma_start(wk_sb[:, :, :], wk.rearrange("p (k c) -> p k c", k=KT))
            for k in range(KT):
                nc.sync.dma_start(xT_sb[:, k, :], xT[:, k * L:(k + 1) * L])
            nc.sync.dma_start(wv_sb[:, :, :], wv.rearrange("p (k c) -> p k c", k=KT))
            wo_sb = sb.tile([128, 2, C], BF16, tag="wo")
            nc.sync.dma_start(wo_sb[:, :, :], wo.rearrange("p (k c) -> p k c", k=2))

            ones_f = sb.tile([128, 64], BF16, tag="ones_f")
            nc.vector.memset(ones_f[:].bitcast(mybir.dt.uint16), 0x3F80)

            qT_sb = sb.tile([128, 2, L], BF16, tag="qT")
            kT_sb = sb.tile([128, 2, L], BF16, tag="kT")

            # ---- QK m=0 projection + V on pp supertiles; m=1 is
            # interleaved into attention units 0-1 (keeps the PE p-state
            # hot through the exp-gated gaps there)
            pp = tc.alloc_tile_pool(name="pp", bufs=2, space="PSUM")

            for w_sb, t_sb in ((wq_sb, qT_sb), (wk_sb, kT_sb)):
                p = pp.tile([128, 2048], F32, tag="qk")
                for k in range(KT):
                    for n in range(4):
                        nc.tensor.matmul(
                            p[:, n * 512:(n + 1) * 512],
                            w_sb[:, k, 0:128],
                            xT_sb[:, k, n * 512:(n + 1) * 512],
                            start=(k == 0), stop=(k == KT - 1),
                        )
                nc.vector.tensor_copy(t_sb[:, 0, :], p[:])

            # V with ones column: v_sb[p, j_tile, h, 0:64]=V, [..., 64]=1
            v_sb = sb.tile([128, 16, H_LOC, D + 1], BF16, tag="v")
            nc.vector.tensor_copy(
                v_sb[:, :, :, D:D + 1],
                ones_f.rearrange("p (a b c) -> p a b c", a=16, b=4),
            )
            for it in range(16):
                p = pp.tile([128, 2048], F32, tag="qk")
                acc = p[:, 0:HD]
                for k in range(KT):
                    nc.tensor.matmul(
                        acc,
                        xT_sb[:, k, it * 128:(it + 1) * 128],
                        wv_sb[:, k, :],
                        start=(k == 0), stop=(k == KT - 1),
                    )
                nc.vector.tensor_copy(
                    v_sb[:, it, :, 0:D],
                    acc.rearrange("p (h d) -> p h d", h=H_LOC),
                )

            pp.release()

            # ---- attention pools: 4 + 2 + 2 = 8 PSUM banks -----------------
            ps = tc.alloc_tile_pool(name="ps", bufs=2, space="PSUM")
            po = tc.alloc_tile_pool(name="po", bufs=1, space="PSUM")
            pw = tc.alloc_tile_pool(name="pw", bufs=1, space="PSUM")

            es_pool = tc.alloc_tile_pool(name="es_pool", bufs=12)
            st_pool = tc.alloc_tile_pool(name="st_pool", bufs=3)
            ost_pool = tc.alloc_tile_pool(name="ost_pool", bufs=6)
            np_pool = tc.alloc_tile_pool(name="np_pool", bufs=3)
            d0_pool = tc.alloc_tile_pool(name="d0_pool", bufs=3)

            oT_sb = sb.tile([128, 2, L], BF16, tag="oT")

            pending = []  # deferred normalize: (m, i0, hl, o_cp, d0)

            def emit_normalize():
                # whole chain on gpsimd so the DVE queue (o_cp/ost copies
                # that gate PSUM ring buffers) never convoys behind the
                # 1.7us partition_broadcast
                m, i0, hl, o_cp, d0 = pending.pop(0)
                rep_sb = st_pool.tile([64, 1024], F32, tag="rep")
                nc.gpsimd.partition_broadcast(rep_sb[:], d0[:])
                with nc.allow_low_precision(reason="bf16 matmul input"):
                    if hl == 0:
                        nc.vector.tensor_mul(
                            oT_sb[0:64, m, i0:i0 + 1024],
                            o_cp[0:64, :], rep_sb[:])
                    else:
                        stage = st_pool.tile([64, 1024], BF16, tag="stage")
                        nc.vector.tensor_mul(
                            stage[:], o_cp[0:64, :], rep_sb[:])
                        nc.gpsimd.dma_start(
                            oT_sb[64:128, m, i0:i0 + 1024], stage[:])

            def emit_o_drain(m, i0, hl, o_h):
                # pull O' off PSUM, reciprocal on the denominator row,
                # defer the PE-side normalize into the next unit's j-loop
                o_cp = np_pool.tile([65, 1024], F32, tag="o_cp")
                nc.vector.tensor_copy(o_cp[:], o_h[:])
                dsq = d0_pool.tile([128, 8], F32, tag="dsq")
                nc.sync.dma_start(dsq[:], o_cp[64:65, :])
                nc.vector.reciprocal(dsq[:], dsq[:])
                d0 = d0_pool.tile([1, 1024], F32, tag="d0")
                nc.sync.dma_start(d0[:], dsq[:])
                pending.append((m, i0, hl, o_cp, d0))

            def emit_wo_half(ih, kk, ct, alt=0):
                # [128, 1024] half-chunk (single kk): 2 matmuls + bf16 stage;
                # kk partials are summed host-side. alt: rotate PSUM pool and
                # copy engine so back-to-back tail chunks overlap.
                i0 = ih * 1024
                pool, tag = (pw, "w") if alt % 2 == 0 else (ps, "s")
                acc = pool.tile([128, 1024], F32, tag=tag, name="wo_ps")
                for n in range(2):
                    nc.tensor.matmul(
                        acc[:, n * 512:(n + 1) * 512],
                        wo_sb[:, kk, ct * 128:(ct + 1) * 128],
                        oT_sb[:, kk, i0 + n * 512:i0 + (n + 1) * 512],
                        start=True, stop=True,
                    )
                ost = ost_pool.tile([128, 1024], BF16, tag="ost", name="ost")
                with nc.allow_low_precision(reason="bf16 output partials"):
                    if alt % 4 < 2:
                        nc.vector.tensor_copy(ost[:], acc[:])
                    else:
                        nc.scalar.copy(ost[:], acc[:])
                nc.sync.dma_start(
                    outTs[kk][ct * 128:(ct + 1) * 128, i0:i0 + 1024], ost[:])

            # wo half-chunk schedule: quarter (ih, kk) ready once both its
            # oT contributors are normalized; spread over later unit j-loops.
            # Quarter q (0..3): (ih, kk) = (q >> 1, q & 1) ready after
            # normalize of units 2q, 2q+1 (popped by unit 2q+2, j==6).
            wo_slots = {}  # (ui, j) -> list of (ih, kk, ct)
            for q in range(4):
                ih, kk = q & 1, q >> 1
                slots = [(2 * q + 2, 9), (2 * q + 2, 13),
                         (2 * q + 3, 1), (2 * q + 3, 5),
                         (2 * q + 3, 9), (2 * q + 3, 13),
                         (2 * q + 4, 1), (2 * q + 4, 5)]
                for ct in range(8):
                    uu, jj = slots[ct]
                    wo_slots.setdefault((uu, jj), []).append((ih, kk, ct))

            # m=1 QK projection: 8 quarter-runs (8 matmuls each)
            # interleaved into units 0-1; keeps the PE warm there
            qk1_runs = [(w_sb, t_sb, nq) for w_sb, t_sb in
                        ((wq_sb, qT_sb), (wk_sb, kT_sb)) for nq in range(4)]

            qk1_state = {}

            def emit_qk1_half(idx):
                # half a quarter-run (4 matmuls, ~0.9us) so the ACT slack
                # absorbs each insertion; the acc spans two slots
                r, half = divmod(idx, 2)
                w_sb, t_sb, nq = qk1_runs[r]
                if half == 0:
                    acc = qk1_state["acc"] = pw.tile(
                        [128, 1024], F32, tag="w", name="qk1")
                    ks = range(0, 4)
                else:
                    acc = qk1_state["acc"]
                    ks = range(4, KT)
                for k in ks:
                    nc.tensor.matmul(
                        acc[:, 0:512],
                        w_sb[:, k, 128:256],
                        xT_sb[:, k, nq * 512:(nq + 1) * 512],
                        start=(k == 0), stop=(k == KT - 1),
                    )
                if half == 1:
                    nc.vector.tensor_copy(
                        t_sb[:, 1, nq * 512:(nq + 1) * 512], acc[:, 0:512])

            qk1_slots = {}
            _qi = 0
            for _u, _js in ((0, (0, 3, 6, 9, 12, 15)),
                            (1, (0, 3, 6, 9, 12, 15)),
                            (2, (0, 3, 6, 15))):
                for _j in _js:
                    qk1_slots[(_u, _j)] = _qi
                    _qi += 1

            # units: pair outer, then i-half, then head-in-pair
            units = [(ih, m, hl) for m in range(2) for ih in range(2)
                     for hl in range(2)]
            units[6], units[7] = units[7], units[6]  # tail unit is hl=0
            tail_wo = []  # chunks whose slot unit index is >= 8

            for key, chunks in list(wo_slots.items()):
                if key[0] >= 8:
                    tail_wo.extend(chunks)
                    del wo_slots[key]

            for ui, (ih, m, hl) in enumerate(units):
                i0 = ih * 1024
                r0 = hl * 64
                h = 2 * m + hl
                of = po.tile([128, 1024], F32, tag="o", name="o_ps")
                o_h = of[0:65, :]
                es_hist = {}
                for j in range(16):
                    if j == 6 and pending:
                        emit_normalize()  # prior unit; reciprocal done by now
                    qs = qk1_slots.get((ui, j))
                    if qs is not None:
                        emit_qk1_half(qs)
                    for ihc, kkc, ctc in wo_slots.get((ui, j), ()):
                        emit_wo_half(ihc, kkc, ctc)
                    s_ps = ps.tile([128, 1024], F32, tag="s", name="s_ps")
                    for n in range(2):
                        nc.tensor.matmul(
                            s_ps[:, n * 512:(n + 1) * 512],
                            kT_sb[r0:r0 + 64, m, j * 128:(j + 1) * 128],
                            qT_sb[r0:r0 + 64, m,
                                  i0 + n * 512:i0 + (n + 1) * 512],
                            start=True, stop=True,
                        )
                    e_sb = es_pool.tile([128, 1024], BF16, tag="es", name="es")
                    nc.scalar.activation(e_sb[:], s_ps[:], AF.Exp, scale=SCALE2)
                    # AV two steps behind: keeps the PE's in-order wait on
                    # exp(j-2) (long done) so S never queues behind a live
                    # exp dependency even with the 2-deep S ring
                    if j >= 2:
                        for n in range(2):
                            nc.tensor.matmul(
                                o_h[:, n * 512:(n + 1) * 512],
                                v_sb[:, j - 2, h, :],
                                es_hist[j - 2][:, n * 512:(n + 1) * 512],
                                start=(j == 2), stop=False,
                            )
                    es_hist[j] = e_sb
                # epilogue AV for j=14, 15
                for jj in (14, 15):
                    for n in range(2):
                        nc.tensor.matmul(
                            o_h[:, n * 512:(n + 1) * 512],
                            v_sb[:, jj, h, :],
                            es_hist[jj][:, n * 512:(n + 1) * 512],
                            start=False, stop=(jj == 15),
                        )
                emit_o_drain(m, i0, hl, o_h)

            while pending:
                emit_normalize()
            for wi, (ihc, kkc, ctc) in enumerate(tail_wo):
                emit_wo_half(ihc, kkc, ctc, alt=wi)

            d0_pool.release()
            np_pool.release()
            ost_pool.release()
            st_pool.release()
            es_pool.release()
            pw.release()
            po.release()
            ps.release()

    nc.compile()
    return nc


def _to_pk(a, kt):
    """[kt*128, c] -> [128, kt*c] host prearrangement for contiguous DMA."""
    c = a.shape[1]
    return np.ascontiguousarray(
        a.reshape(kt, 128, c).transpose(1, 0, 2).reshape(128, kt * c))


def _prep_in_maps(x, Wq, Wk, Wv, Wo):
    xTs = [_to_pk(np.ascontiguousarray(x[b].T), KT).astype(NPBF16)
           for b in range(B)]
    in_maps = []
    for core in range(8):
        b, hg = divmod(core, 4)
        sl = slice(hg * HD, (hg + 1) * HD)
        in_maps.append({
            "xT": xTs[b],
            "wq": _to_pk(np.ascontiguousarray(Wq[:, sl]), KT).astype(NPBF16),
            "wk": _to_pk(np.ascontiguousarray(Wk[:, sl]), KT).astype(NPBF16),
            "wv": _to_pk(np.ascontiguousarray(Wv[:, sl]), KT).astype(NPBF16),
            "wo": _to_pk(np.ascontiguousarray(Wo[sl, :]), 2).astype(NPBF16),
        })
    return in_maps


def kernel(x, Wq, Wk, Wv, Wo, bo):
    x = np.asarray(x, dtype=np.float32)
    Wq = np.asarray(Wq, dtype=np.float32)
    Wk = np.asarray(Wk, dtype=np.float32)
    Wv = np.asarray(Wv, dtype=np.float32)
    Wo = np.asarray(Wo, dtype=np.float32)
    bo = np.asarray(bo, dtype=np.float32)

    if "nc" not in _cache:
        _cache["nc"] = _build()
    nc = _cache["nc"]

    in_maps = _prep_in_maps(x, Wq, Wk, Wv, Wo)
    globals()["_last_in_maps"] = in_maps

    res = run_bass_kernel_spmd(nc, in_maps, core_ids=list(range(8)))
    out = np.empty((B, L, C), dtype=np.float32)
    for b in range(B):
        acc = res.results[4 * b]["outT0"].astype(np.float32)
        acc = acc + res.results[4 * b]["outT1"].astype(np.float32)
        for hg in range(1, 4):
            acc = acc + res.results[4 * b + hg]["outT0"].astype(np.float32)
            acc = acc + res.results[4 * b + hg]["outT1"].astype(np.float32)
        out[b] = acc.T + bo
    return out
